# revision 1
# baseline (speedup 1.0000x reference)
"""Trainium2 Bass kernel for nn_ExperimentNet (SE-style pooling net).

Reference computation (per batch b):
    pool = mean(x[b], axis=(H,W))                # (C,)
    f    = sigmoid(relu(pool @ W1.T) @ W2.T)     # (C,)
    p    = mean(x[b] * f[:,None,None], (H,W))    # (C,)  == f * pool  (f const over H,W)
    out  = p @ W3.T + b3                         # (2,)

Key algebraic identity: mean(x * f) over (H,W) equals f * mean(x), so x is
read exactly ONCE (512 MB total).  Everything after the pooling is a tiny
MLP on (B, C) = (32, 256) values.

Strategy: pure data parallel over 8 NeuronCores, 4 batches per core.
Per core: stream the (4*256, 16384) row-major shard through SBUF, reduce
over the free (spatial) dim on DVE, then run the whole MLP on-chip.

Tail minimization (the DMA stream itself is at the ~358 GB/s per-NC HBM
roofline, so the only wins left are at the edges):
  * groups are streamed c-major (all channel-chunk-0 groups first), so the
    f1 = W1 @ pool contraction over the first 128 channels runs mid-stream;
  * the whole MLP is pipelined per batch COLUMN: batch b's chain (f1 c=1
    matmul -> relu -> f2 -> sigmoid -> *pool -> out-psum) is emitted right
    after its last pool-reduce, so after the final x byte only batch 3's
    tiny chain remains;
  * the final group's last chunk shrinks geometrically (2048,1024,512,256,
    256 cols), so the last reduce covers 256 cols instead of 4096 and the
    two halves go on DVE and ACT in parallel;
  * the output is produced transposed (2, B_LOC) so the +b3 bias folds into
    the single ACT copy (per-partition bias), saving a DVE hop.

The 1/(H*W) mean scaling is folded into host-prepared W1.T and W3.T copies
(exact: 16384 is a power of two), so the kernel only ever needs raw sums.
"""

import numpy as np

import concourse.bacc as bacc
import concourse.bass as bass
import concourse.mybir as mybir
from concourse import tile
from concourse.bass_utils import run_bass_kernel_spmd

N_CORES = 8
B, C, H, W = 32, 256, 128, 128
S = H * W                  # 16384 spatial elements per (b, c)
B_LOC = B // N_CORES       # 4 batches per core
ROWS = B_LOC * C           # 1024 (b, c) rows per core
P = 128                    # SBUF partitions
G = ROWS // P              # 8 row groups per core
CR = C // 4                # 64 hidden units
KC = C // P                # 2 contraction chunks of 128 for C-dim matmuls

FP32 = mybir.dt.float32

_CACHE = {}


def _build_nc(ch=8192, bufs=6, act_frac=0.5, tail_geo=True, tail_min=512,
              reps=1, serialize_reps=True, dual_ring=False, rings=None,
              loop_reps=0, tail_par=True,
              no_mlp=False, empty=False, half_all=False):
    """Build the per-core bass program.

    ch: free-dim chunk per DMA; bufs: xin double-buffer depth;
    act_frac: fraction of chunk reductions routed to ScalarE (ACT) instead
    of VectorE (DVE); tail_geo: shrink the final group's last chunk
    geometrically down to tail_min cols so the last reduce is tiny.
    reps / serialize_reps / loop_reps: benchmarking-only repetition (see
    test.py; each For_i back-edge is a full all-engine barrier).
    """
    nch = S // ch
    nc = bacc.Bacc("TRN2", target_bir_lowering=False, debug=False)
    if rings is None:
        rings = ["sync", "scalar"] if dual_ring else ["sync"]

    x_d = nc.dram_tensor("x", [ROWS, S], FP32, kind="ExternalInput")
    w1t_d = nc.dram_tensor("w1t", [C, CR], FP32, kind="ExternalInput")   # W1.T / S
    w2t_d = nc.dram_tensor("w2t", [CR, C], FP32, kind="ExternalInput")   # W2.T
    w3t_d = nc.dram_tensor("w3t", [C, 2], FP32, kind="ExternalInput")    # W3.T / S
    b3b_d = nc.dram_tensor("b3b", [2, 1], FP32, kind="ExternalInput")
    out_d = nc.dram_tensor("out", [2, B_LOC], FP32, kind="ExternalOutput")

    with tile.TileContext(nc) as tc:
        with (
            tc.tile_pool(name="xin", bufs=bufs) as xpool,
            tc.tile_pool(name="small", bufs=1) as spool,
            tc.tile_pool(name="stage", bufs=4) as stpool,
            tc.tile_pool(name="psum", bufs=1, space="PSUM") as ppool,
        ):
            # --- persistent small tiles -------------------------------------
            # Weight loads go on the ACT HWDGE ring so they don't delay the
            # x-stream at the head of the sync ring's FIFO.
            w_eng = nc.scalar
            w1t = []
            w3t = []
            for c in range(KC):
                t1 = spool.tile([P, CR], FP32, tag=f"w1t{c}", name=f"w1t{c}")
                w_eng.dma_start(t1[:], w1t_d[c * P:(c + 1) * P, :])
                w1t.append(t1)
                t3 = spool.tile([P, 2], FP32, tag=f"w3t{c}", name=f"w3t{c}")
                w_eng.dma_start(t3[:], w3t_d[c * P:(c + 1) * P, :])
                w3t.append(t3)
            w2t = spool.tile([CR, C], FP32, tag="w2t")
            w_eng.dma_start(w2t[:], w2t_d[:])
            b3b = spool.tile([2, 1], FP32, tag="b3b")
            w_eng.dma_start(b3b[:], b3b_d[:])

            def body(rep):
                if empty:
                    # timing diagnostic: barrier-only loop body
                    z = spool.tile([2, B_LOC], FP32, tag="resT")
                    nc.vector.tensor_scalar_mul(z[:], w2t[0:2, 0:B_LOC], 0.0)
                    return
                # poolT[c][p, b] = sum over spatial of x[b, c*128+p, :, :]
                poolT = [
                    spool.tile([P, B_LOC], FP32, tag=f"poolT{c}",
                               name=f"poolT{c}_{rep}")
                    for c in range(KC)
                ]
                ps_f1 = ppool.tile([CR, B_LOC], FP32, tag="ps_f1")
                ps_f2 = [
                    ppool.tile([P, B_LOC], FP32, tag=f"ps_f2{c}",
                               name=f"ps_f2{c}_{rep}")
                    for c in range(KC)
                ]
                ps_oT = ppool.tile([2, B_LOC], FP32, tag="ps_oT")
                f1 = spool.tile([CR, B_LOC], FP32, tag="f1")
                f2 = [
                    spool.tile([P, B_LOC], FP32, tag=f"f2{c}",
                               name=f"f2{c}_{rep}")
                    for c in range(KC)
                ]
                pT = [
                    spool.tile([P, B_LOC], FP32, tag=f"pT{c}",
                               name=f"pT{c}_{rep}")
                    for c in range(KC)
                ]
                resT = spool.tile([2, B_LOC], FP32, tag="resT")

                # --- streaming reduction over x, c-major group order --------
                act_acc = 0.0
                dma_i = 0
                for c_idx in range(KC):
                    for b_idx in range(B_LOC):
                        g = b_idx * KC + c_idx   # row-group in x layout
                        last_group = (c_idx == KC - 1 and b_idx == B_LOC - 1)
                        pieces = [(j * ch, ch) for j in range(nch)]
                        if last_group and tail_geo:
                            st0, w0 = pieces.pop()
                            off, rem = st0, w0
                            wsub = w0 // 2
                            while wsub >= tail_min and rem - wsub >= tail_min:
                                pieces.append((off, wsub))
                                off += wsub
                                rem -= wsub
                                wsub //= 2
                            pieces.append((off, rem))
                        split = half_all or (last_group and tail_par)
                        n_cols = 2 * len(pieces) if split else len(pieces)
                        stage = stpool.tile([P, n_cols], FP32, tag="stage")
                        for j, (col0, width) in enumerate(pieces):
                            xt = xpool.tile([P, width], FP32, tag="xt")
                            dma_eng = getattr(nc, rings[dma_i % len(rings)])
                            dma_i += 1
                            dma_eng.dma_start(
                                xt[:],
                                x_d[g * P:(g + 1) * P, col0:col0 + width],
                            )
                            if split:
                                # Critical-path group: reduce every piece's
                                # two halves on DVE and ACT in parallel so
                                # neither engine serializes the tail.
                                half = width // 2
                                nc.vector.reduce_sum(
                                    stage[:, 2 * j:2 * j + 1], xt[:, :half],
                                    axis=mybir.AxisListType.X,
                                )
                                nc.scalar.activation(
                                    xt[:, half:], xt[:, half:],
                                    mybir.ActivationFunctionType.Copy,
                                    accum_out=stage[:, 2 * j + 1:2 * j + 2],
                                )
                                continue
                            act_acc += act_frac
                            if act_acc >= 1.0:
                                act_acc -= 1.0
                                nc.scalar.activation(
                                    xt[:], xt[:],
                                    mybir.ActivationFunctionType.Copy,
                                    accum_out=stage[:, j:j + 1],
                                )
                            else:
                                nc.vector.reduce_sum(
                                    stage[:, j:j + 1], xt[:],
                                    axis=mybir.AxisListType.X,
                                )
                        bb = b_idx
                        nc.vector.reduce_sum(
                            poolT[c_idx][:, bb:bb + 1], stage[:],
                            axis=mybir.AxisListType.X,
                        )

                        # --- per-batch-column MLP pipelining ----------------
                        if no_mlp:
                            continue
                        if c_idx == 0:
                            # first half of the f1 contraction, mid-stream
                            nc.tensor.matmul(
                                ps_f1[:, bb:bb + 1], w1t[0][:],
                                poolT[0][:, bb:bb + 1],
                                start=True, stop=False,
                            )
                        else:
                            nc.tensor.matmul(
                                ps_f1[:, bb:bb + 1], w1t[1][:],
                                poolT[1][:, bb:bb + 1],
                                start=False, stop=True,
                            )
                            nc.scalar.activation(
                                f1[:, bb:bb + 1], ps_f1[:, bb:bb + 1],
                                mybir.ActivationFunctionType.Relu,
                            )
                            for c in range(KC):
                                nc.tensor.matmul(
                                    ps_f2[c][:, bb:bb + 1],
                                    w2t[:, c * P:(c + 1) * P],
                                    f1[:, bb:bb + 1],
                                    start=True, stop=True,
                                )
                                nc.scalar.activation(
                                    f2[c][:, bb:bb + 1],
                                    ps_f2[c][:, bb:bb + 1],
                                    mybir.ActivationFunctionType.Sigmoid,
                                )
                                nc.vector.tensor_mul(
                                    pT[c][:, bb:bb + 1],
                                    f2[c][:, bb:bb + 1],
                                    poolT[c][:, bb:bb + 1],
                                )
                                nc.tensor.matmul(
                                    ps_oT[:, bb:bb + 1], w3t[c][:],
                                    pT[c][:, bb:bb + 1],
                                    start=(c == 0), stop=(c == KC - 1),
                                )
                            # resT col = ps_oT col + b3 (per-partition bias);
                            # emitted per batch so only batch 3's is in the
                            # post-stream tail
                            nc.scalar.activation(
                                resT[:, bb:bb + 1], ps_oT[:, bb:bb + 1],
                                mybir.ActivationFunctionType.Identity,
                                bias=b3b[:],
                            )

                if no_mlp:
                    # timing diagnostic: stream+reduce only, dummy output
                    nc.vector.tensor_scalar_mul(
                        resT[:], w2t[0:2, 0:B_LOC], 0.0)
                nc.scalar.dma_start(out_d[:], resT[:])

            if loop_reps:
                # Dynamic loop for benchmarking: each back-edge is a full
                # all-engine barrier (+ sem reset), so iterations serialize
                # like independent executions.  Tiny NEFF, huge device time.
                with tc.For_i(0, loop_reps, 1):
                    body(0)
            else:
                for rep in range(reps):
                    if rep > 0 and serialize_reps:
                        tc.strict_bb_all_engine_barrier()
                    body(rep)

    nc.compile()
    return nc


def _get_nc(**kw):
    key = tuple(sorted(kw.items()))
    if key not in _CACHE:
        _CACHE[key] = _build_nc(**kw)
    return _CACHE[key]


def kernel(x, W1, W2, W3, b3, **_unused):
    x = np.ascontiguousarray(np.asarray(x, dtype=np.float32))
    w1t = (np.asarray(W1, np.float32).T / np.float32(S)).astype(np.float32)
    w1t = np.ascontiguousarray(w1t)                       # (C, CR)
    w2t = np.ascontiguousarray(np.asarray(W2, np.float32).T)   # (CR, C)
    w3t = np.ascontiguousarray(
        (np.asarray(W3, np.float32).T / np.float32(S)).astype(np.float32)
    )                                                     # (C, 2)
    b3b = np.ascontiguousarray(
        np.asarray(b3, np.float32).reshape(2, 1)
    )

    nc = _get_nc()
    in_maps = [
        {
            "x": x[i * B_LOC:(i + 1) * B_LOC].reshape(ROWS, S),
            "w1t": w1t,
            "w2t": w2t,
            "w3t": w3t,
            "b3b": b3b,
        }
        for i in range(N_CORES)
    ]
    res = run_bass_kernel_spmd(nc, in_maps, list(range(N_CORES)))
    # per-core out is transposed (2, B_LOC); batch b = core*B_LOC + col
    out = np.concatenate(
        [res.results[i]["out"].T for i in range(N_CORES)], axis=0
    )
    return out.astype(np.float32)



# revision 22
# speedup vs baseline: 1.0023x; 1.0023x over previous
"""Trainium2 Bass kernel for nn_ExperimentNet (SE-style pooling net).

Reference computation (per batch b):
    pool = mean(x[b], axis=(H,W))                # (C,)
    f    = sigmoid(relu(pool @ W1.T) @ W2.T)     # (C,)
    p    = mean(x[b] * f[:,None,None], (H,W))    # (C,)  == f * pool  (f const over H,W)
    out  = p @ W3.T + b3                         # (2,)

Key algebraic identity: mean(x * f) over (H,W) equals f * mean(x), so x is
read exactly ONCE (512 MB total).  Everything after the pooling is a tiny
MLP on (B, C) = (32, 256) values.

Strategy: pure data parallel over 8 NeuronCores, 4 batches per core.
Per core: stream the (4*256, 16384) row-major shard through SBUF, reduce
over the free (spatial) dim on DVE, then run the whole MLP on-chip.

Tail minimization (the DMA stream itself is at the ~358 GB/s per-NC HBM
roofline, so the only wins left are at the edges):
  * groups are streamed c-major (all channel-chunk-0 groups first), so the
    f1 = W1 @ pool contraction over the first 128 channels runs mid-stream;
  * the whole MLP is pipelined per batch COLUMN: batch b's chain (f1 c=1
    matmul -> relu -> f2 -> sigmoid -> *pool -> out-psum) is emitted right
    after its last pool-reduce, so after the final x byte only batch 3's
    tiny chain remains;
  * the final group's last chunk shrinks geometrically (2048,1024,512,256,
    256 cols), so the last reduce covers 256 cols instead of 4096 and the
    two halves go on DVE and ACT in parallel;
  * the output is produced transposed (2, B_LOC) so the +b3 bias folds into
    the single ACT copy (per-partition bias), saving a DVE hop.

The 1/(H*W) mean scaling is folded into host-prepared W1.T and W3.T copies
(exact: 16384 is a power of two), so the kernel only ever needs raw sums.
"""

import numpy as np

import concourse.bacc as bacc
import concourse.bass as bass
import concourse.mybir as mybir
from concourse import tile
from concourse.bass_utils import run_bass_kernel_spmd

N_CORES = 8
B, C, H, W = 32, 256, 128, 128
S = H * W                  # 16384 spatial elements per (b, c)
B_LOC = B // N_CORES       # 4 batches per core
ROWS = B_LOC * C           # 1024 (b, c) rows per core
P = 128                    # SBUF partitions
G = ROWS // P              # 8 row groups per core
CR = C // 4                # 64 hidden units
KC = C // P                # 2 contraction chunks of 128 for C-dim matmuls

FP32 = mybir.dt.float32

_CACHE = {}


def _build_nc(ch=8192, bufs=6, act_frac=0.5, tail_geo=True, tail_min=512,
              reps=1, serialize_reps=True, dual_ring=False, rings=None,
              loop_reps=0, tail_par=True,
              no_mlp=False, empty=False, half_all=False,
              packed_w=False, act_mul=False, piece_acc=False,
              dve_tail=False, f2col=False, sp_out=False,
              tail_first_dve=0, last_dve=False):
    """Build the per-core bass program.

    ch: free-dim chunk per DMA; bufs: xin double-buffer depth;
    act_frac: fraction of chunk reductions routed to ScalarE (ACT) instead
    of VectorE (DVE); tail_geo: shrink the final group's last chunk
    geometrically down to tail_min cols so the last reduce is tiny.
    packed_w: all weights ride in ONE [128, 261] DRAM tensor / one DMA.
    act_mul: the f2*pool gating runs on ACT (scale-AP mul) instead of DVE,
    dropping two cross-engine hops from the final batch's tail chain.
    piece_acc: the last group's f1 contraction accumulates per PIECE on PE
    (each stage column matmul-accumulated as it lands), so relu does not
    wait for the full-group stage reduce; the stage reduce (needed for the
    pool gate) runs in parallel with the relu/f2 chain.
    reps / serialize_reps / loop_reps: benchmarking-only repetition (see
    test.py; each For_i back-edge is a full all-engine barrier).
    """
    nch = S // ch
    nc = bacc.Bacc("TRN2", target_bir_lowering=False, debug=False)
    if rings is None:
        rings = ["sync", "scalar"] if dual_ring else ["sync"]

    x_d = nc.dram_tensor("x", [ROWS, S], FP32, kind="ExternalInput")
    if packed_w:
        # w1t c0|c1 (cols 0:128), w3t c0|c1 (cols 128:132), b3 at
        # partitions 0:2 col 132 — one DMA.  w2t stays separate (its chunk
        # slices must share base partition 0 with the f1 rhs).
        wpk_d = nc.dram_tensor("wpk", [P, 133], FP32, kind="ExternalInput")
        w2t_d = nc.dram_tensor("w2t", [CR, C], FP32, kind="ExternalInput")
    else:
        w1t_d = nc.dram_tensor("w1t", [C, CR], FP32, kind="ExternalInput")   # W1.T / S
        w2t_d = nc.dram_tensor("w2t", [CR, C], FP32, kind="ExternalInput")   # W2.T
        w3t_d = nc.dram_tensor("w3t", [C, 2], FP32, kind="ExternalInput")    # W3.T / S
        b3b_d = nc.dram_tensor("b3b", [2, 1], FP32, kind="ExternalInput")
    out_d = nc.dram_tensor("out", [2, B_LOC], FP32, kind="ExternalOutput")

    with tile.TileContext(nc) as tc:
        with (
            tc.tile_pool(name="xin", bufs=bufs) as xpool,
            tc.tile_pool(name="small", bufs=1) as spool,
            tc.tile_pool(name="stage", bufs=4) as stpool,
            tc.tile_pool(name="psum", bufs=1, space="PSUM") as ppool,
        ):
            # --- persistent small tiles -------------------------------------
            # Weight loads go on the ACT HWDGE ring so they don't delay the
            # x-stream at the head of the sync ring's FIFO.
            w_eng = nc.scalar
            if packed_w:
                wpk = spool.tile([P, 133], FP32, tag="wpk")
                w_eng.dma_start(wpk[:], wpk_d[:])
                w2t = spool.tile([CR, C], FP32, tag="w2t")
                w_eng.dma_start(w2t[:], w2t_d[:])
                w1t = [wpk[:, 0:CR], wpk[:, CR:2 * CR]]
                w2t_c = [w2t[:, c * P:(c + 1) * P] for c in range(KC)]
                w3t = [wpk[:, 128:130], wpk[:, 130:132]]
                b3b = wpk[0:2, 132:133]
            else:
                w1t = []
                w3t = []
                for c in range(KC):
                    t1 = spool.tile([P, CR], FP32, tag=f"w1t{c}", name=f"w1t{c}")
                    w_eng.dma_start(t1[:], w1t_d[c * P:(c + 1) * P, :])
                    w1t.append(t1[:])
                    t3 = spool.tile([P, 2], FP32, tag=f"w3t{c}", name=f"w3t{c}")
                    w_eng.dma_start(t3[:], w3t_d[c * P:(c + 1) * P, :])
                    w3t.append(t3[:])
                w2t = spool.tile([CR, C], FP32, tag="w2t")
                w_eng.dma_start(w2t[:], w2t_d[:])
                w2t_c = [w2t[:, c * P:(c + 1) * P] for c in range(KC)]
                b3b_t = spool.tile([2, 1], FP32, tag="b3b")
                w_eng.dma_start(b3b_t[:], b3b_d[:])
                b3b = b3b_t[:]

            def body(rep):
                dummy = wpk if packed_w else w2t
                if empty:
                    # timing diagnostic: barrier-only loop body
                    z = spool.tile([2, B_LOC], FP32, tag="resT")
                    nc.vector.tensor_scalar_mul(z[:], dummy[0:2, 0:B_LOC], 0.0)
                    return
                # poolT[c][p, b] = sum over spatial of x[b, c*128+p, :, :]
                poolT = [
                    spool.tile([P, B_LOC], FP32, tag=f"poolT{c}",
                               name=f"poolT{c}_{rep}")
                    for c in range(KC)
                ]
                ps_f1 = ppool.tile([CR, B_LOC], FP32, tag="ps_f1")
                if f2col:
                    # both c-chunks of a batch adjacent -> ONE sigmoid op
                    ps_f2w = ppool.tile([P, 2 * B_LOC], FP32, tag="ps_f2w",
                                        name=f"ps_f2w_{rep}")
                    f2w = spool.tile([P, 2 * B_LOC], FP32, tag="f2w",
                                     name=f"f2w_{rep}")
                else:
                    ps_f2 = [
                        ppool.tile([P, B_LOC], FP32, tag=f"ps_f2{c}",
                                   name=f"ps_f2{c}_{rep}")
                        for c in range(KC)
                    ]
                    f2 = [
                        spool.tile([P, B_LOC], FP32, tag=f"f2{c}",
                                   name=f"f2{c}_{rep}")
                        for c in range(KC)
                    ]
                ps_oT = ppool.tile([2, B_LOC], FP32, tag="ps_oT")
                f1 = spool.tile([CR, B_LOC], FP32, tag="f1")
                pT = [
                    spool.tile([P, B_LOC], FP32, tag=f"pT{c}",
                               name=f"pT{c}_{rep}")
                    for c in range(KC)
                ]
                resT = spool.tile([2, B_LOC], FP32, tag="resT")

                # --- streaming reduction over x, c-major group order --------
                act_acc = 0.0
                dma_i = 0
                for c_idx in range(KC):
                    for b_idx in range(B_LOC):
                        g = b_idx * KC + c_idx   # row-group in x layout
                        last_group = (c_idx == KC - 1 and b_idx == B_LOC - 1)
                        pieces = [(j * ch, ch) for j in range(nch)]
                        if last_group and tail_geo:
                            st0, w0 = pieces.pop()
                            off, rem = st0, w0
                            wsub = w0 // 2
                            while wsub >= tail_min and rem - wsub >= tail_min:
                                pieces.append((off, wsub))
                                off += wsub
                                rem -= wsub
                                wsub //= 2
                            pieces.append((off, rem))
                        tail_dve = dve_tail and last_group
                        split = (half_all or (last_group and tail_par)) \
                            and not tail_dve
                        acc_here = piece_acc and last_group and not no_mlp \
                            and (split or tail_dve)
                        n_cols = 2 * len(pieces) if split else len(pieces)
                        stage = stpool.tile([P, n_cols], FP32, tag="stage")
                        for j, (col0, width) in enumerate(pieces):
                            xt = xpool.tile([P, width], FP32, tag="xt")
                            dma_eng = getattr(nc, rings[dma_i % len(rings)])
                            dma_i += 1
                            dma_eng.dma_start(
                                xt[:],
                                x_d[g * P:(g + 1) * P, col0:col0 + width],
                            )
                            if tail_dve:
                                # Last group: every piece reduce on DVE so
                                # ACT's queue is empty when the MLP chain
                                # starts; f1 accumulates per piece on PE.
                                nc.vector.reduce_sum(
                                    stage[:, j:j + 1], xt[:],
                                    axis=mybir.AxisListType.X,
                                )
                                if acc_here:
                                    nc.tensor.matmul(
                                        ps_f1[:, b_idx:b_idx + 1], w1t[1],
                                        stage[:, j:j + 1],
                                        start=False, stop=j == len(pieces) - 1,
                                    )
                                continue
                            if split and last_group and last_dve \
                                    and j == len(pieces) - 1:
                                # final piece: single DVE reduce (ACT's
                                # 187ns accum-read tax and one extra PE
                                # matmul would otherwise sit on the
                                # critical path)
                                nc.vector.reduce_sum(
                                    stage[:, 2 * j:2 * j + 1], xt[:],
                                    axis=mybir.AxisListType.X,
                                )
                                nc.vector.tensor_scalar_mul(
                                    stage[:, 2 * j + 1:2 * j + 2],
                                    stage[:, 2 * j:2 * j + 1], 0.0,
                                )
                                if acc_here:
                                    nc.tensor.matmul(
                                        ps_f1[:, b_idx:b_idx + 1], w1t[1],
                                        stage[:, 2 * j:2 * j + 1],
                                        start=False, stop=True,
                                    )
                                continue
                            if split and last_group and j < tail_first_dve:
                                # big leading piece(s): whole-piece DVE
                                # reduce, keeping ACT's tail queue short
                                nc.vector.reduce_sum(
                                    stage[:, 2 * j:2 * j + 1], xt[:],
                                    axis=mybir.AxisListType.X,
                                )
                                nc.vector.tensor_scalar_mul(
                                    stage[:, 2 * j + 1:2 * j + 2],
                                    stage[:, 2 * j:2 * j + 1], 0.0,
                                )
                                if acc_here:
                                    nc.tensor.matmul(
                                        ps_f1[:, b_idx:b_idx + 1], w1t[1],
                                        stage[:, 2 * j:2 * j + 1],
                                        start=False, stop=False,
                                    )
                                continue
                            if split:
                                # Critical-path group: reduce every piece's
                                # two halves on DVE and ACT in parallel so
                                # neither engine serializes the tail.
                                half = width // 2
                                nc.vector.reduce_sum(
                                    stage[:, 2 * j:2 * j + 1], xt[:, :half],
                                    axis=mybir.AxisListType.X,
                                )
                                nc.scalar.activation(
                                    xt[:, half:], xt[:, half:],
                                    mybir.ActivationFunctionType.Copy,
                                    accum_out=stage[:, 2 * j + 1:2 * j + 2],
                                )
                                if acc_here:
                                    # f1's c=1 contraction is linear in the
                                    # piece sums: accumulate each stage col
                                    # into ps_f1 on PE as it lands, so relu
                                    # is not gated on the full stage reduce.
                                    lastj = j == len(pieces) - 1
                                    nc.tensor.matmul(
                                        ps_f1[:, b_idx:b_idx + 1], w1t[1],
                                        stage[:, 2 * j:2 * j + 1],
                                        start=False, stop=False,
                                    )
                                    nc.tensor.matmul(
                                        ps_f1[:, b_idx:b_idx + 1], w1t[1],
                                        stage[:, 2 * j + 1:2 * j + 2],
                                        start=False, stop=lastj,
                                    )
                                continue
                            act_acc += act_frac
                            if act_acc >= 1.0:
                                act_acc -= 1.0
                                nc.scalar.activation(
                                    xt[:], xt[:],
                                    mybir.ActivationFunctionType.Copy,
                                    accum_out=stage[:, j:j + 1],
                                )
                            else:
                                nc.vector.reduce_sum(
                                    stage[:, j:j + 1], xt[:],
                                    axis=mybir.AxisListType.X,
                                )
                        bb = b_idx
                        nc.vector.reduce_sum(
                            poolT[c_idx][:, bb:bb + 1], stage[:],
                            axis=mybir.AxisListType.X,
                        )

                        # --- per-batch-column MLP pipelining ----------------
                        if no_mlp:
                            continue
                        if c_idx == 0:
                            # first half of the f1 contraction, mid-stream
                            nc.tensor.matmul(
                                ps_f1[:, bb:bb + 1], w1t[0][:],
                                poolT[0][:, bb:bb + 1],
                                start=True, stop=False,
                            )
                        else:
                            if not acc_here:
                                nc.tensor.matmul(
                                    ps_f1[:, bb:bb + 1], w1t[1],
                                    poolT[1][:, bb:bb + 1],
                                    start=False, stop=True,
                                )
                            nc.scalar.activation(
                                f1[:, bb:bb + 1], ps_f1[:, bb:bb + 1],
                                mybir.ActivationFunctionType.Relu,
                            )
                            if f2col:
                                for c in range(KC):
                                    nc.tensor.matmul(
                                        ps_f2w[:, 2 * bb + c:2 * bb + c + 1],
                                        w2t_c[c],
                                        f1[:, bb:bb + 1],
                                        start=True, stop=True,
                                    )
                                nc.scalar.activation(
                                    f2w[:, 2 * bb:2 * bb + 2],
                                    ps_f2w[:, 2 * bb:2 * bb + 2],
                                    mybir.ActivationFunctionType.Sigmoid,
                                )
                            f2v = (
                                [f2w[:, 2 * bb + c:2 * bb + c + 1]
                                 for c in range(KC)]
                                if f2col else None
                            )
                            for c in range(KC):
                                if not f2col:
                                    nc.tensor.matmul(
                                        ps_f2[c][:, bb:bb + 1],
                                        w2t_c[c],
                                        f1[:, bb:bb + 1],
                                        start=True, stop=True,
                                    )
                                    nc.scalar.activation(
                                        f2[c][:, bb:bb + 1],
                                        ps_f2[c][:, bb:bb + 1],
                                        mybir.ActivationFunctionType.Sigmoid,
                                    )
                                f2ap = f2v[c] if f2col \
                                    else f2[c][:, bb:bb + 1]
                                if act_mul:
                                    # per-partition-scalar mul on ACT: no
                                    # ACT->DVE->PE round trip in the tail
                                    nc.scalar.mul(
                                        pT[c][:, bb:bb + 1],
                                        f2ap,
                                        poolT[c][:, bb:bb + 1],
                                    )
                                else:
                                    nc.vector.tensor_mul(
                                        pT[c][:, bb:bb + 1],
                                        f2ap,
                                        poolT[c][:, bb:bb + 1],
                                    )
                                nc.tensor.matmul(
                                    ps_oT[:, bb:bb + 1], w3t[c],
                                    pT[c][:, bb:bb + 1],
                                    start=(c == 0), stop=(c == KC - 1),
                                )
                            # resT col = ps_oT col + b3 (per-partition bias);
                            # emitted per batch so only batch 3's is in the
                            # post-stream tail
                            nc.scalar.activation(
                                resT[:, bb:bb + 1], ps_oT[:, bb:bb + 1],
                                mybir.ActivationFunctionType.Identity,
                                bias=b3b,
                            )

                if no_mlp:
                    # timing diagnostic: stream+reduce only, dummy output
                    nc.vector.tensor_scalar_mul(
                        resT[:], dummy[0:2, 0:B_LOC], 0.0)
                out_eng = nc.sync if sp_out else nc.scalar
                out_eng.dma_start(out_d[:], resT[:])

            if loop_reps:
                # Dynamic loop for benchmarking: each back-edge is a full
                # all-engine barrier (+ sem reset), so iterations serialize
                # like independent executions.  Tiny NEFF, huge device time.
                with tc.For_i(0, loop_reps, 1):
                    body(0)
            else:
                for rep in range(reps):
                    if rep > 0 and serialize_reps:
                        tc.strict_bb_all_engine_barrier()
                    body(rep)

    nc.compile()
    return nc


def _get_nc(**kw):
    key = tuple(sorted(kw.items()))
    if key not in _CACHE:
        _CACHE[key] = _build_nc(**kw)
    return _CACHE[key]


# Default build configuration for kernel() and test.py's bench variants.
DEFAULT_KW = dict(packed_w=True, act_mul=True, piece_acc=True, sp_out=True,
                  last_dve=True, tail_min=256)


def make_in_maps(x, W1, W2, W3, b3, kw=None):
    """Per-core input dicts matching _build_nc(**kw)'s dram tensors."""
    kw = DEFAULT_KW if kw is None else kw
    x = np.ascontiguousarray(np.asarray(x, dtype=np.float32))
    w1t = np.ascontiguousarray(
        (np.asarray(W1, np.float32).T / np.float32(S)).astype(np.float32)
    )                                                     # (C, CR)
    w2t = np.ascontiguousarray(np.asarray(W2, np.float32).T)   # (CR, C)
    w3t = np.ascontiguousarray(
        (np.asarray(W3, np.float32).T / np.float32(S)).astype(np.float32)
    )                                                     # (C, 2)
    b3v = np.asarray(b3, np.float32)
    if kw.get("packed_w"):
        wpk = np.zeros((P, 133), np.float32)
        wpk[:, 0:CR] = w1t[0:P]
        wpk[:, CR:2 * CR] = w1t[P:2 * P]
        wpk[:, 128:130] = w3t[0:P]
        wpk[:, 130:132] = w3t[P:2 * P]
        wpk[0:2, 132] = b3v
        consts = {"wpk": np.ascontiguousarray(wpk), "w2t": w2t}
    else:
        consts = {
            "w1t": w1t,
            "w2t": w2t,
            "w3t": w3t,
            "b3b": np.ascontiguousarray(b3v.reshape(2, 1)),
        }
    return [
        {"x": x[i * B_LOC:(i + 1) * B_LOC].reshape(ROWS, S), **consts}
        for i in range(N_CORES)
    ]


def kernel(x, W1, W2, W3, b3, **_unused):
    nc = _get_nc(**DEFAULT_KW)
    in_maps = make_in_maps(x, W1, W2, W3, b3)
    res = run_bass_kernel_spmd(nc, in_maps, list(range(N_CORES)))
    # per-core out is transposed (2, B_LOC); batch b = core*B_LOC + col
    out = np.concatenate(
        [res.results[i]["out"].T for i in range(N_CORES)], axis=0
    )
    return out.astype(np.float32)



# revision 37
# speedup vs baseline: 1.0505x; 1.0481x over previous
"""Trainium2 Bass kernel for nn_ExperimentNet (SE-style pooling net).

Reference computation (per batch b):
    pool = mean(x[b], axis=(H,W))                # (C,)
    f    = sigmoid(relu(pool @ W1.T) @ W2.T)     # (C,)
    p    = mean(x[b] * f[:,None,None], (H,W))    # (C,)  == f * pool  (f const over H,W)
    out  = p @ W3.T + b3                         # (2,)

Key algebraic identity: mean(x * f) over (H,W) equals f * mean(x), so x is
read exactly ONCE (512 MB total).  Everything after the pooling is a tiny
MLP on (B, C) = (32, 256) values.

Strategy: pure data parallel over 8 NeuronCores, 4 batches per core.
Per core: stream the (4*256, 16384) row-major shard through SBUF, reduce
over the free (spatial) dim on DVE, then run the whole MLP on-chip.

HW-measured facts this version is tuned on (loop-slope protocol, quiet
machine, within-run A/B):
  * the DMA stream sustains ~341-345 GB/s per NC (not the ~358 GB/s HBM
    share) and is the hard floor: pure-DMA body (no compute) measures
    ~195.9 us/iter.  1 MiB DMAs (ch=2048) stream ~2 us faster than 4 MiB
    (ch=8192); 0.5 MiB is no better and hurts the full kernel.
  * the For_i back-edge barrier costs ~1.1 us/iter (empty-body slope).
  * the post-stream MLP tail costs ~2.4 us on HW (~3x the cost model's
    estimate: real cross-engine sem hops are ~0.3-0.5 us each).

Tail minimization:
  * groups are streamed c-major (all channel-chunk-0 groups first), so the
    f1 = W1 @ pool contraction over the first 128 channels runs mid-stream;
  * the whole MLP is pipelined per batch COLUMN: batch b's chain (f1 c=1
    matmul -> relu -> f2 -> sigmoid -> *pool -> out-psum) is emitted right
    after its last pool-reduce, so after the final x byte only batch 3's
    tiny chain remains;
  * the final group's last chunk shrinks geometrically (1024,512,512 cols)
    and the final piece is reduced by a single DVE op (last_dve), with
    f1's c=1 contraction matmul-accumulated per piece on PE (piece_acc) so
    relu is gated only on the last tiny piece, not the full-group reduce;
  * the f2*pool gate runs on ACT via a scale-AP mul (act_mul), dropping
    the ACT->DVE->PE round trip from the tail;
  * w1t/w3t/b3 ride in one packed [128,133] tensor -> one weight DMA
    (packed_w);
  * the output is produced transposed (2, B_LOC) so the +b3 bias folds into
    the single ACT copy (per-partition bias).

The 1/(H*W) mean scaling is folded into host-prepared W1.T and W3.T copies
(exact: 16384 is a power of two), so the kernel only ever needs raw sums.
"""

import numpy as np

import concourse.bacc as bacc
import concourse.bass as bass
import concourse.mybir as mybir
from concourse import tile
from concourse.bass_utils import run_bass_kernel_spmd

N_CORES = 8
B, C, H, W = 32, 256, 128, 128
S = H * W                  # 16384 spatial elements per (b, c)
B_LOC = B // N_CORES       # 4 batches per core
ROWS = B_LOC * C           # 1024 (b, c) rows per core
P = 128                    # SBUF partitions
G = ROWS // P              # 8 row groups per core
CR = C // 4                # 64 hidden units
KC = C // P                # 2 contraction chunks of 128 for C-dim matmuls

FP32 = mybir.dt.float32

_CACHE = {}


def _build_nc(ch=8192, bufs=6, act_frac=0.5, tail_geo=True, tail_min=512,
              reps=1, serialize_reps=True, dual_ring=False, rings=None,
              loop_reps=0, tail_par=True,
              no_mlp=False, empty=False, half_all=False,
              packed_w=False, act_mul=False, piece_acc=False,
              dve_tail=False, f2col=False, sp_out=False,
              tail_first_dve=0, last_dve=False, w3ps=False, tiled=False,
              no_reduce=False):
    """Build the per-core bass program.

    ch: free-dim chunk per DMA; bufs: xin double-buffer depth;
    act_frac: fraction of chunk reductions routed to ScalarE (ACT) instead
    of VectorE (DVE); tail_geo: shrink the final group's last chunk
    geometrically down to tail_min cols so the last reduce is tiny.
    packed_w: all weights ride in ONE [128, 261] DRAM tensor / one DMA.
    act_mul: the f2*pool gating runs on ACT (scale-AP mul) instead of DVE,
    dropping two cross-engine hops from the final batch's tail chain.
    piece_acc: the last group's f1 contraction accumulates per PIECE on PE
    (each stage column matmul-accumulated as it lands), so relu does not
    wait for the full-group stage reduce; the stage reduce (needed for the
    pool gate) runs in parallel with the relu/f2 chain.
    reps / serialize_reps / loop_reps: benchmarking-only repetition (see
    test.py; each For_i back-edge is a full all-engine barrier).
    """
    nch = S // ch
    nc = bacc.Bacc("TRN2", target_bir_lowering=False, debug=False)
    if rings is None:
        rings = ["sync", "scalar"] if dual_ring else ["sync"]

    if tiled:
        # host pre-tiles the shard so the device reads one monotonically
        # increasing contiguous 64 MiB: tile t = stream-order index, each
        # [128, ch] tile contiguous (row stride = ch*4 bytes)
        x_d = nc.dram_tensor("x", [G * (S // ch) * P, ch], FP32,
                             kind="ExternalInput")
    else:
        x_d = nc.dram_tensor("x", [ROWS, S], FP32, kind="ExternalInput")
    if packed_w:
        # w1t c0|c1 (cols 0:128), w3t c0|c1 (cols 128:132), b3 at
        # partitions 0:2 col 132 — one DMA.  w2t stays separate (its chunk
        # slices must share base partition 0 with the f1 rhs).
        wpk_d = nc.dram_tensor("wpk", [P, 133], FP32, kind="ExternalInput")
        w2t_d = nc.dram_tensor("w2t", [CR, C], FP32, kind="ExternalInput")
    else:
        w1t_d = nc.dram_tensor("w1t", [C, CR], FP32, kind="ExternalInput")   # W1.T / S
        w2t_d = nc.dram_tensor("w2t", [CR, C], FP32, kind="ExternalInput")   # W2.T
        w3t_d = nc.dram_tensor("w3t", [C, 2], FP32, kind="ExternalInput")    # W3.T / S
        b3b_d = nc.dram_tensor("b3b", [2, 1], FP32, kind="ExternalInput")
    out_d = nc.dram_tensor("out", [2, B_LOC], FP32, kind="ExternalOutput")

    with tile.TileContext(nc) as tc:
        with (
            tc.tile_pool(name="xin", bufs=bufs) as xpool,
            tc.tile_pool(name="small", bufs=1) as spool,
            tc.tile_pool(name="stage", bufs=4) as stpool,
            tc.tile_pool(name="psum", bufs=1, space="PSUM") as ppool,
        ):
            # --- persistent small tiles -------------------------------------
            # Weight loads go on the ACT HWDGE ring so they don't delay the
            # x-stream at the head of the sync ring's FIFO.
            w_eng = nc.scalar
            if packed_w:
                wpk = spool.tile([P, 133], FP32, tag="wpk")
                w_eng.dma_start(wpk[:], wpk_d[:])
                w2t = spool.tile([CR, C], FP32, tag="w2t")
                w_eng.dma_start(w2t[:], w2t_d[:])
                w1t = [wpk[:, 0:CR], wpk[:, CR:2 * CR]]
                w2t_c = [w2t[:, c * P:(c + 1) * P] for c in range(KC)]
                w3t = [wpk[:, 128:130], wpk[:, 130:132]]
                b3b = wpk[0:2, 132:133]
            else:
                w1t = []
                w3t = []
                for c in range(KC):
                    t1 = spool.tile([P, CR], FP32, tag=f"w1t{c}", name=f"w1t{c}")
                    w_eng.dma_start(t1[:], w1t_d[c * P:(c + 1) * P, :])
                    w1t.append(t1[:])
                    t3 = spool.tile([P, 2], FP32, tag=f"w3t{c}", name=f"w3t{c}")
                    w_eng.dma_start(t3[:], w3t_d[c * P:(c + 1) * P, :])
                    w3t.append(t3[:])
                w2t = spool.tile([CR, C], FP32, tag="w2t")
                w_eng.dma_start(w2t[:], w2t_d[:])
                w2t_c = [w2t[:, c * P:(c + 1) * P] for c in range(KC)]
                b3b_t = spool.tile([2, 1], FP32, tag="b3b")
                w_eng.dma_start(b3b_t[:], b3b_d[:])
                b3b = b3b_t[:]

            def body(rep):
                dummy = wpk if packed_w else w2t
                if empty:
                    # timing diagnostic: barrier-only loop body
                    z = spool.tile([2, B_LOC], FP32, tag="resT")
                    nc.vector.tensor_scalar_mul(z[:], dummy[0:2, 0:B_LOC], 0.0)
                    return
                # poolT[c][p, b] = sum over spatial of x[b, c*128+p, :, :]
                poolT = [
                    spool.tile([P, B_LOC], FP32, tag=f"poolT{c}",
                               name=f"poolT{c}_{rep}")
                    for c in range(KC)
                ]
                ps_f1 = ppool.tile([CR, B_LOC], FP32, tag="ps_f1")
                if f2col:
                    # both c-chunks of a batch adjacent -> ONE sigmoid op
                    ps_f2w = ppool.tile([P, 2 * B_LOC], FP32, tag="ps_f2w",
                                        name=f"ps_f2w_{rep}")
                    f2w = spool.tile([P, 2 * B_LOC], FP32, tag="f2w",
                                     name=f"f2w_{rep}")
                else:
                    ps_f2 = [
                        ppool.tile([P, B_LOC], FP32, tag=f"ps_f2{c}",
                                   name=f"ps_f2{c}_{rep}")
                        for c in range(KC)
                    ]
                    f2 = [
                        spool.tile([P, B_LOC], FP32, tag=f"f2{c}",
                                   name=f"f2{c}_{rep}")
                        for c in range(KC)
                    ]
                ps_oT = ppool.tile([2, B_LOC], FP32, tag="ps_oT")
                f1 = spool.tile([CR, B_LOC], FP32, tag="f1")
                pT = [
                    spool.tile([P, B_LOC], FP32, tag=f"pT{c}",
                               name=f"pT{c}_{rep}")
                    for c in range(KC)
                ]
                resT = spool.tile([2, B_LOC], FP32, tag="resT")
                w3p = [[None] * KC for _ in range(B_LOC)]
                if w3ps:
                    # w3 prescaled by the pool gate (DVE, off critical
                    # path): out = (w3t*pool).T @ sigmoid(ps_f2), removing
                    # the gate muls from the tail chain
                    for bb_ in range(B_LOC):
                        for c_ in range(KC):
                            w3p[bb_][c_] = spool.tile(
                                [P, 2], FP32, tag=f"w3p{bb_}_{c_}",
                                name=f"w3p{bb_}_{c_}_{rep}",
                            )

                # --- streaming reduction over x, c-major group order --------
                act_acc = 0.0
                dma_i = 0
                for c_idx in range(KC):
                    for b_idx in range(B_LOC):
                        g = b_idx * KC + c_idx   # row-group in x layout
                        last_group = (c_idx == KC - 1 and b_idx == B_LOC - 1)
                        pieces = [(j * ch, ch) for j in range(nch)]
                        if last_group and tail_geo:
                            st0, w0 = pieces.pop()
                            off, rem = st0, w0
                            wsub = w0 // 2
                            while wsub >= tail_min and rem - wsub >= tail_min:
                                pieces.append((off, wsub))
                                off += wsub
                                rem -= wsub
                                wsub //= 2
                            pieces.append((off, rem))
                        tail_dve = dve_tail and last_group
                        split = (half_all or (last_group and tail_par)) \
                            and not tail_dve
                        acc_here = piece_acc and last_group and not no_mlp \
                            and (split or tail_dve)
                        n_cols = 2 * len(pieces) if split else len(pieces)
                        stage = stpool.tile([P, n_cols], FP32, tag="stage")
                        for j, (col0, width) in enumerate(pieces):
                            xt = xpool.tile([P, width], FP32, tag="xt")
                            dma_eng = getattr(nc, rings[dma_i % len(rings)])
                            dma_i += 1
                            if tiled:
                                t = ((c_idx * B_LOC + b_idx) * nch
                                     + col0 // ch)
                                src = x_d[t * P:(t + 1) * P,
                                          col0 % ch:col0 % ch + width]
                            else:
                                src = x_d[g * P:(g + 1) * P,
                                          col0:col0 + width]
                            dma_eng.dma_start(xt[:], src)
                            if no_reduce:
                                # diagnostic: pure-DMA stream, tiles never
                                # consumed (pool recycles on WAR ordering)
                                continue
                            if tail_dve:
                                # Last group: every piece reduce on DVE so
                                # ACT's queue is empty when the MLP chain
                                # starts; f1 accumulates per piece on PE.
                                nc.vector.reduce_sum(
                                    stage[:, j:j + 1], xt[:],
                                    axis=mybir.AxisListType.X,
                                )
                                if acc_here:
                                    nc.tensor.matmul(
                                        ps_f1[:, b_idx:b_idx + 1], w1t[1],
                                        stage[:, j:j + 1],
                                        start=False, stop=j == len(pieces) - 1,
                                    )
                                continue
                            if split and last_group and last_dve \
                                    and j == len(pieces) - 1:
                                # final piece: single DVE reduce (ACT's
                                # 187ns accum-read tax and one extra PE
                                # matmul would otherwise sit on the
                                # critical path)
                                nc.vector.reduce_sum(
                                    stage[:, 2 * j:2 * j + 1], xt[:],
                                    axis=mybir.AxisListType.X,
                                )
                                nc.vector.tensor_scalar_mul(
                                    stage[:, 2 * j + 1:2 * j + 2],
                                    stage[:, 2 * j:2 * j + 1], 0.0,
                                )
                                if acc_here:
                                    nc.tensor.matmul(
                                        ps_f1[:, b_idx:b_idx + 1], w1t[1],
                                        stage[:, 2 * j:2 * j + 1],
                                        start=False, stop=True,
                                    )
                                continue
                            if split and last_group and j < tail_first_dve:
                                # big leading piece(s): whole-piece DVE
                                # reduce, keeping ACT's tail queue short
                                nc.vector.reduce_sum(
                                    stage[:, 2 * j:2 * j + 1], xt[:],
                                    axis=mybir.AxisListType.X,
                                )
                                nc.vector.tensor_scalar_mul(
                                    stage[:, 2 * j + 1:2 * j + 2],
                                    stage[:, 2 * j:2 * j + 1], 0.0,
                                )
                                if acc_here:
                                    nc.tensor.matmul(
                                        ps_f1[:, b_idx:b_idx + 1], w1t[1],
                                        stage[:, 2 * j:2 * j + 1],
                                        start=False, stop=False,
                                    )
                                continue
                            if split:
                                # Critical-path group: reduce every piece's
                                # two halves on DVE and ACT in parallel so
                                # neither engine serializes the tail.
                                half = width // 2
                                nc.vector.reduce_sum(
                                    stage[:, 2 * j:2 * j + 1], xt[:, :half],
                                    axis=mybir.AxisListType.X,
                                )
                                nc.scalar.activation(
                                    xt[:, half:], xt[:, half:],
                                    mybir.ActivationFunctionType.Copy,
                                    accum_out=stage[:, 2 * j + 1:2 * j + 2],
                                )
                                if acc_here:
                                    # f1's c=1 contraction is linear in the
                                    # piece sums: accumulate each stage col
                                    # into ps_f1 on PE as it lands, so relu
                                    # is not gated on the full stage reduce.
                                    lastj = j == len(pieces) - 1
                                    nc.tensor.matmul(
                                        ps_f1[:, b_idx:b_idx + 1], w1t[1],
                                        stage[:, 2 * j:2 * j + 1],
                                        start=False, stop=False,
                                    )
                                    nc.tensor.matmul(
                                        ps_f1[:, b_idx:b_idx + 1], w1t[1],
                                        stage[:, 2 * j + 1:2 * j + 2],
                                        start=False, stop=lastj,
                                    )
                                continue
                            act_acc += act_frac
                            if act_acc >= 1.0:
                                act_acc -= 1.0
                                nc.scalar.activation(
                                    xt[:], xt[:],
                                    mybir.ActivationFunctionType.Copy,
                                    accum_out=stage[:, j:j + 1],
                                )
                            else:
                                nc.vector.reduce_sum(
                                    stage[:, j:j + 1], xt[:],
                                    axis=mybir.AxisListType.X,
                                )
                        if no_reduce:
                            continue
                        bb = b_idx
                        nc.vector.reduce_sum(
                            poolT[c_idx][:, bb:bb + 1], stage[:],
                            axis=mybir.AxisListType.X,
                        )

                        # --- per-batch-column MLP pipelining ----------------
                        if no_mlp:
                            continue
                        if c_idx == 0:
                            # first half of the f1 contraction, mid-stream
                            nc.tensor.matmul(
                                ps_f1[:, bb:bb + 1], w1t[0],
                                poolT[0][:, bb:bb + 1],
                                start=True, stop=False,
                            )
                            if w3ps:
                                nc.vector.tensor_scalar_mul(
                                    w3p[bb][0][:], w3t[0],
                                    poolT[0][:, bb:bb + 1],
                                )
                        else:
                            if w3ps:
                                # DVE, in parallel with the relu/f2 chain
                                nc.vector.tensor_scalar_mul(
                                    w3p[bb][1][:], w3t[1],
                                    poolT[1][:, bb:bb + 1],
                                )
                            if not acc_here:
                                nc.tensor.matmul(
                                    ps_f1[:, bb:bb + 1], w1t[1],
                                    poolT[1][:, bb:bb + 1],
                                    start=False, stop=True,
                                )
                            nc.scalar.activation(
                                f1[:, bb:bb + 1], ps_f1[:, bb:bb + 1],
                                mybir.ActivationFunctionType.Relu,
                            )
                            if f2col:
                                for c in range(KC):
                                    nc.tensor.matmul(
                                        ps_f2w[:, 2 * bb + c:2 * bb + c + 1],
                                        w2t_c[c],
                                        f1[:, bb:bb + 1],
                                        start=True, stop=True,
                                    )
                                nc.scalar.activation(
                                    f2w[:, 2 * bb:2 * bb + 2],
                                    ps_f2w[:, 2 * bb:2 * bb + 2],
                                    mybir.ActivationFunctionType.Sigmoid,
                                )
                            f2v = (
                                [f2w[:, 2 * bb + c:2 * bb + c + 1]
                                 for c in range(KC)]
                                if f2col else None
                            )
                            for c in range(KC):
                                if not f2col:
                                    nc.tensor.matmul(
                                        ps_f2[c][:, bb:bb + 1],
                                        w2t_c[c],
                                        f1[:, bb:bb + 1],
                                        start=True, stop=True,
                                    )
                                    nc.scalar.activation(
                                        f2[c][:, bb:bb + 1],
                                        ps_f2[c][:, bb:bb + 1],
                                        mybir.ActivationFunctionType.Sigmoid,
                                    )
                                f2ap = f2v[c] if f2col \
                                    else f2[c][:, bb:bb + 1]
                                if w3ps:
                                    nc.tensor.matmul(
                                        ps_oT[:, bb:bb + 1], w3p[bb][c][:],
                                        f2ap,
                                        start=(c == 0), stop=(c == KC - 1),
                                    )
                                    continue
                                if act_mul:
                                    # per-partition-scalar mul on ACT: no
                                    # ACT->DVE->PE round trip in the tail
                                    nc.scalar.mul(
                                        pT[c][:, bb:bb + 1],
                                        f2ap,
                                        poolT[c][:, bb:bb + 1],
                                    )
                                else:
                                    nc.vector.tensor_mul(
                                        pT[c][:, bb:bb + 1],
                                        f2ap,
                                        poolT[c][:, bb:bb + 1],
                                    )
                                nc.tensor.matmul(
                                    ps_oT[:, bb:bb + 1], w3t[c],
                                    pT[c][:, bb:bb + 1],
                                    start=(c == 0), stop=(c == KC - 1),
                                )
                            # resT col = ps_oT col + b3 (per-partition bias);
                            # emitted per batch so only batch 3's is in the
                            # post-stream tail
                            nc.scalar.activation(
                                resT[:, bb:bb + 1], ps_oT[:, bb:bb + 1],
                                mybir.ActivationFunctionType.Identity,
                                bias=b3b,
                            )

                if no_mlp or no_reduce:
                    # timing diagnostic: stream+reduce only, dummy output
                    nc.vector.tensor_scalar_mul(
                        resT[:], dummy[0:2, 0:B_LOC], 0.0)
                out_eng = nc.sync if sp_out else nc.scalar
                out_eng.dma_start(out_d[:], resT[:])

            if loop_reps:
                # Dynamic loop for benchmarking: each back-edge is a full
                # all-engine barrier (+ sem reset), so iterations serialize
                # like independent executions.  Tiny NEFF, huge device time.
                with tc.For_i(0, loop_reps, 1):
                    body(0)
            else:
                for rep in range(reps):
                    if rep > 0 and serialize_reps:
                        tc.strict_bb_all_engine_barrier()
                    body(rep)

    nc.compile()
    return nc


def _get_nc(**kw):
    key = tuple(sorted(
        (k, tuple(v) if isinstance(v, list) else v) for k, v in kw.items()
    ))
    if key not in _CACHE:
        _CACHE[key] = _build_nc(**kw)
    return _CACHE[key]


# Default build configuration for kernel() and test.py's bench variants.
# HW-tuned (within-run A/B on quiet machine, loop-slope protocol):
#   ch=2048/bufs=24: the 1 MiB-per-DMA stream runs ~2 us faster than the
#     4 MiB ch=8192 stream on HW (pure-DMA A/B: 195.9 vs 198.1 us);
#   act_frac=0.25: DVE takes 3/4 of the mid-stream chunk reduces since ACT
#     also runs the per-batch MLP chains;
#   packed_w/act_mul/piece_acc/last_dve: tail-chain restructuring, ~0.5 us.
DEFAULT_KW = dict(packed_w=True, act_mul=True, piece_acc=True,
                  last_dve=True, act_frac=0.25, ch=2048, bufs=24)


def make_in_maps(x, W1, W2, W3, b3, kw=None):
    """Per-core input dicts matching _build_nc(**kw)'s dram tensors."""
    kw = DEFAULT_KW if kw is None else kw
    x = np.ascontiguousarray(np.asarray(x, dtype=np.float32))
    w1t = np.ascontiguousarray(
        (np.asarray(W1, np.float32).T / np.float32(S)).astype(np.float32)
    )                                                     # (C, CR)
    w2t = np.ascontiguousarray(np.asarray(W2, np.float32).T)   # (CR, C)
    w3t = np.ascontiguousarray(
        (np.asarray(W3, np.float32).T / np.float32(S)).astype(np.float32)
    )                                                     # (C, 2)
    b3v = np.asarray(b3, np.float32)
    if kw.get("packed_w"):
        wpk = np.zeros((P, 133), np.float32)
        wpk[:, 0:CR] = w1t[0:P]
        wpk[:, CR:2 * CR] = w1t[P:2 * P]
        wpk[:, 128:130] = w3t[0:P]
        wpk[:, 130:132] = w3t[P:2 * P]
        wpk[0:2, 132] = b3v
        consts = {"wpk": np.ascontiguousarray(wpk), "w2t": w2t}
    else:
        consts = {
            "w1t": w1t,
            "w2t": w2t,
            "w3t": w3t,
            "b3b": np.ascontiguousarray(b3v.reshape(2, 1)),
        }
    def _shard(i):
        xs = x[i * B_LOC:(i + 1) * B_LOC].reshape(ROWS, S)
        if kw.get("tiled"):
            ch_v = kw.get("ch", 8192)
            nch = S // ch_v
            # tile order (c, b, j) = device stream order, tiles contiguous
            xs = np.ascontiguousarray(
                xs.reshape(B_LOC, KC, P, nch, ch_v)
                .transpose(1, 0, 3, 2, 4)
                .reshape(G * nch * P, ch_v)
            )
        return xs

    return [{"x": _shard(i), **consts} for i in range(N_CORES)]


def kernel(x, W1, W2, W3, b3, **_unused):
    nc = _get_nc(**DEFAULT_KW)
    in_maps = make_in_maps(x, W1, W2, W3, b3)
    res = run_bass_kernel_spmd(nc, in_maps, list(range(N_CORES)))
    # per-core out is transposed (2, B_LOC); batch b = core*B_LOC + col
    out = np.concatenate(
        [res.results[i]["out"].T for i in range(N_CORES)], axis=0
    )
    return out.astype(np.float32)

